# revision 26
# baseline (speedup 1.0000x reference)
"""Sliding-window attention (window = [i-128, i+128]) on 8 TRN2 NeuronCores.

Problem: B=4, L=4096, D=256, fp32.  out = softmax(mask(Q K^T / sqrt(256))) V
with the softmax restricted to keys j in [i-128, i+128] for query i.

Sharding (no collectives): core c handles (batch b = c//2, query-half
h = c%2) -> 2048 queries with a 2304-row K/V halo slab; rows outside
[0, L) are zero-padded and carry a 0 validity indicator that removes them
from the softmax denominator.

Per-core kernel, S^T layout (keys on partitions, queries on free dim):
  - 8 superblocks of 256 queries; each sees a 512-key window (4 chunks of
    128 keys). Chunk 0 is only valid for queries r<128 and chunk 3 only
    for r>=128, so they share one PSUM slot ("folded" layout) and mm1
    computes only their valid query halves.
  - mm1: S^T = K Q^T accumulated over 2 d-chunks into PSUM [128, 3, 256].
  - exp: one ACT pass Exp(S^T / 16) -> SBUF (bf16/f32 per variant).
  - band mask (keep iff 0 <= jf - r <= 256): 4 GPSIMD affine_selects.
  - mm2: O = sum_chunks P_chunk^T @ V_chunk; V carries an extra indicator
    column whose matmul output is the softmax denominator.
  - normalize: DVE reciprocal + broadcast multiply (second subblock on
    the ACT engine to balance load); outputs batched 2 superblocks per
    DMA (last two solo for a faster tail).

Measured on trn2 (8 cores, NTFF profile): ~38 us NEFF exec, scale-
relative max abs error 4.4e-3 vs the fp32 reference (bf16 variant).
KERNEL_VARIANT=f32r gives 2.2e-4 error at ~63 us; f32 gives 3.4e-6 at
~83 us.
"""

import os

import numpy as np

import concourse.bass as bass  # noqa: F401  (engine types via nc)
import concourse.mybir as mybir
import concourse.tile as tile
from concourse import bacc
from concourse.bass_utils import run_bass_kernel_spmd

B = 4
L = 4096
D = 256
LW = 128                 # window half-width
N_CORES = 8
QS = L // 2              # queries per core
KS = QS + 2 * LW         # k/v slab rows per core
SB = 256                 # superblock query count
NSB = QS // SB           # superblocks per core
NKC = KS // 128          # key chunks per core
VW = 258                 # V width: 256 data + 1 indicator + 1 pad

_F32 = mybir.dt.float32

VARIANT = os.environ.get("KERNEL_VARIANT", "bf16")

# input piece boundaries for streamed DMA (few, large, arrival-ordered)
QT_PIECES = [(0, 1), (1, 2), (2, 4), (4, 6), (6, 8)]      # superblock ranges
KV_PIECES = [(0, 4), (4, 8), (8, 12), (12, 15), (15, 18)]  # key-chunk ranges


def build_bass(variant=VARIANT):
    """variant: 'f32' (exact), 'f32r' (fp32 bits, fast reduced-precision
    matmul), 'bf16' (bf16 inputs + probabilities, fp32 accumulation)."""
    if variant == "bf16":
        mm_dtype = mybir.dt.bfloat16
    elif variant == "f32r":
        mm_dtype = mybir.dt.float32r
    else:
        mm_dtype = _F32

    nc = bacc.Bacc(
        "TRN2", target_bir_lowering=False, debug=False, num_devices=N_CORES
    )
    qT = nc.declare_dram_parameter("qT", [128, NSB, 2, SB], mm_dtype, isOutput=False)
    kT = nc.declare_dram_parameter("kT", [128, NKC, 2, 128], mm_dtype, isOutput=False)
    vA = nc.declare_dram_parameter("vA", [128, NKC, VW], mm_dtype, isOutput=False)
    out = nc.declare_dram_parameter("out", [128, QS // 128, D], _F32, isOutput=True)

    inv_sqrt_d = float(1.0 / np.sqrt(D))

    with tile.TileContext(nc) as tc:
        with (
            tc.tile_pool(name="res", bufs=1) as res,
            tc.tile_pool(name="work", bufs=4) as work,
            tc.tile_pool(name="outp", bufs=2) as outp,
            tc.tile_pool(name="rcp", bufs=4) as rcp,
            tc.tile_pool(name="ps_s", bufs=2, space="PSUM") as ps_s,
            tc.tile_pool(name="ps_o", bufs=4, space="PSUM") as ps_o,
        ):
            qT_sb = res.tile([128, NSB, 2, SB], mm_dtype)
            kT_sb = res.tile([128, NKC, 2, 128], mm_dtype)
            vA_sb = res.tile([128, NKC, VW], mm_dtype)

            # streamed input DMA (~0.65us issue cost per instruction,
            # FIFO per HWDGE ring): mm1 operands interleaved on the sync
            # ring, mm2 operands (vA, needed later) on the scalar ring.
            first = True
            for (k0, k1), (s0, s1) in zip(KV_PIECES, QT_PIECES):
                if first:
                    # qT piece 0 gates the first real matmul - issue it first
                    nc.sync.dma_start(qT_sb[:, s0:s1], qT[:, s0:s1])
                    nc.sync.dma_start(kT_sb[:, k0:k1], kT[:, k0:k1])
                    first = False
                else:
                    nc.sync.dma_start(kT_sb[:, k0:k1], kT[:, k0:k1])
                    nc.sync.dma_start(qT_sb[:, s0:s1], qT[:, s0:s1])
            for k0, k1 in [(0, 3), (3, 8), (8, 13), (13, 18)]:
                nc.scalar.dma_start(vA_sb[:, k0:k1], vA[:, k0:k1])

            # PE warm-up: dummy matmuls while input DMAs land, so the HAM
            # clock-gate reaches 8/8 soon after the real matmuls start;
            # the real stream continues the busy window seamlessly.
            warm_t = res.tile([128, 128], mm_dtype)
            nc.vector.memset(warm_t[:], 0.0)
            warm_ps = ps_o.tile([128, 128], _F32, tag="psum_o")
            for _ in range(60):
                nc.tensor.matmul(
                    warm_ps[:, 0:64], lhsT=warm_t[:], rhs=warm_t[:, 0:64],
                    start=True, stop=True,
                )

            # Reusable band masks: m0 covers the folded slot0 (chunk0 for
            # r<128, chunk3 for r>=128) applied by GPSIMD affine_selects
            # in-place on exp_s; m12 covers slots 1+2, applied as ONE DVE
            # multiply [128, 2, SB]. Mask conditions (keep iff >= 0):
            #  slot0[:, 0:128]  (chunk0): p - r
            #  slot0[:, 128:]   (chunk3): (r-128) - p
            #  slot1 (chunk1): 128 + p - r
            #  slot2 (chunk2): r - p
            m12 = res.tile([128, 2, SB], mm_dtype)
            nc.vector.memset(m12[:], 1.0)
            for (slot, sl), base, cm, step, n in [
                ((0, slice(0, SB)), 128, 1, -1, SB),
                ((1, slice(0, SB)), 0, -1, 1, SB),
            ]:
                nc.gpsimd.affine_select(
                    out=m12[:, slot, sl],
                    in_=m12[:, slot, sl],
                    compare_op=mybir.AluOpType.is_ge,
                    fill=0.0,
                    base=base,
                    channel_multiplier=cm,
                    pattern=[[step, n]],
                )

            def emit_mm1_exp(s):
                # mm1 into folded PSUM [128, 3, SB]: slot0 holds chunk0
                # (queries 0:128) and chunk3 (queries 128:256).
                psum_s = ps_s.tile([128, 3, SB], _F32)
                for jj, q_sl, slot, p_sl in [
                    (0, slice(0, 128), 0, slice(0, 128)),        # chunk 0
                    (1, slice(0, SB), 1, slice(0, SB)),          # chunk 1
                    (2, slice(0, SB), 2, slice(0, SB)),          # chunk 2
                    (3, slice(128, SB), 0, slice(128, SB)),      # chunk 3
                ]:
                    jc = 2 * s + jj
                    for dc in range(2):
                        nc.tensor.matmul(
                            psum_s[:, slot, p_sl],
                            lhsT=kT_sb[:, jc, dc, :],
                            rhs=qT_sb[:, s, dc, q_sl],
                            start=(dc == 0),
                            stop=(dc == 1),
                        )

                # exp over the folded window in one ACT pass, then band
                # masks: folded slot0 halves on GPSIMD, slots 1+2 as one
                # DVE multiply (GPSIMD and DVE run concurrently).
                exp_s = work.tile([128, 3, SB], mm_dtype)
                nc.scalar.activation(
                    exp_s[:],
                    psum_s[:],
                    mybir.ActivationFunctionType.Exp,
                    scale=inv_sqrt_d,
                )
                for (slot, sl), base, cm, step, n in [
                    ((0, slice(0, 128)), 0, 1, -1, 128),
                    ((0, slice(128, SB)), 0, -1, 1, 128),
                ]:
                    nc.gpsimd.affine_select(
                        out=exp_s[:, slot, sl],
                        in_=exp_s[:, slot, sl],
                        compare_op=mybir.AluOpType.is_ge,
                        fill=0.0,
                        base=base,
                        channel_multiplier=cm,
                        pattern=[[step, n]],
                    )
                nc.vector.tensor_mul(
                    out=exp_s[:, 1:3, :], in0=exp_s[:, 1:3, :], in1=m12[:]
                )
                return exp_s

            o_hold = {}

            def emit_mm2(s, exp_s):
                single_out = s >= 6  # faster tail: last 2 superblocks solo
                if single_out:
                    o_sb = outp.tile([128, 2, D], _F32, tag="o_single")
                elif s % 2 == 0:
                    o_sb = outp.tile([128, 4, D], _F32, tag="o_pair")
                    o_hold[0] = o_sb
                else:
                    o_sb = o_hold.pop(0)
                for qc in range(2):
                    if qc == 0:
                        parts = [(0, 0, slice(0, 128)), (1, 1, slice(0, 128)),
                                 (2, 2, slice(0, 128))]
                    else:
                        parts = [(1, 1, slice(128, SB)), (2, 2, slice(128, SB)),
                                 (3, 0, slice(128, SB))]
                    psum_o = ps_o.tile([128, VW], _F32)
                    for i, (jj, slot, q_sl) in enumerate(parts):
                        nc.tensor.matmul(
                            psum_o[:],
                            lhsT=exp_s[:, slot, q_sl],
                            rhs=vA_sb[:, 2 * s + jj, :],
                            start=(i == 0),
                            stop=(i == 2),
                        )
                    recip = rcp.tile([128, 1], _F32)
                    nc.vector.reciprocal(recip[:], psum_o[:, 256:257])
                    oc = qc if single_out else 2 * (s % 2) + qc
                    if qc == 0:
                        nc.vector.tensor_scalar_mul(
                            o_sb[:, oc, :], psum_o[:, 0:D], recip[:]
                        )
                    else:
                        # second subblock normalized on ACT to offload DVE
                        nc.scalar.mul(o_sb[:, oc, :], psum_o[:, 0:D], recip[:])
                if single_out:
                    t0 = 2 * s
                    nc.sync.dma_start(out[:, t0 : t0 + 2, :], o_sb[:])
                elif s % 2 == 1:
                    t0 = 2 * (s - 1)
                    nc.sync.dma_start(out[:, t0 : t0 + 4, :], o_sb[:])

            # depth-2 software pipeline: PE FIFO runs mm1(s) two
            # superblocks ahead of mm2(s), so the exp->mask chain (ACT +
            # GPSIMD/DVE, ~1.6us) never stalls the PE.
            exp_tiles = {}
            for s in range(NSB):
                exp_tiles[s] = emit_mm1_exp(s)
                if s >= 2:
                    emit_mm2(s - 2, exp_tiles.pop(s - 2))
            emit_mm2(NSB - 2, exp_tiles.pop(NSB - 2))
            emit_mm2(NSB - 1, exp_tiles.pop(NSB - 1))

    nc.compile()
    return nc


def make_in_maps(query, key, value, np_dtype=np.float32):
    """Host-side shard + transpose + pad. Returns list of 8 input dicts."""
    in_maps = []
    for c in range(N_CORES):
        b, h = c // 2, c % 2
        q0 = h * QS
        qc = np.asarray(query[b, q0 : q0 + QS, :], dtype=np.float32)
        # qT[p, s, dc, r] = qc[SB*s + r, 128*dc + p]
        qT = np.ascontiguousarray(
            qc.reshape(NSB, SB, 2, 128).transpose(3, 0, 2, 1)
        ).astype(np_dtype)

        kstart = q0 - LW
        lo, hi = max(0, kstart), min(L, kstart + KS)
        kp = np.zeros((KS, D), np.float32)
        kp[lo - kstart : hi - kstart] = key[b, lo:hi]
        # kT[p, jc, dc, j] = kp[128*jc + j, 128*dc + p]
        kT = np.ascontiguousarray(
            kp.reshape(NKC, 128, 2, 128).transpose(3, 0, 2, 1)
        ).astype(np_dtype)

        va = np.zeros((KS, VW), np.float32)
        va[lo - kstart : hi - kstart, :D] = value[b, lo:hi]
        va[lo - kstart : hi - kstart, D] = 1.0
        vA = np.ascontiguousarray(
            va.reshape(NKC, 128, VW).transpose(1, 0, 2)
        ).astype(np_dtype)

        in_maps.append({"qT": qT, "kT": kT, "vA": vA})
    return in_maps


_NC_CACHE = {}


def _get_nc():
    if "nc" not in _NC_CACHE:
        _NC_CACHE["nc"] = build_bass(VARIANT)
    return _NC_CACHE["nc"]


def _np_in_dtype():
    if VARIANT == "bf16":
        import ml_dtypes

        return ml_dtypes.bfloat16
    return np.float32


def kernel(query, key, value):
    nc = _get_nc()
    in_maps = make_in_maps(query, key, value, np_dtype=_np_in_dtype())
    res = run_bass_kernel_spmd(nc, in_maps, core_ids=list(range(N_CORES)))
    out = np.empty((B, L, D), np.float32)
    for c in range(N_CORES):
        b, h = c // 2, c % 2
        oc = res.results[c]["out"]  # [128, QS//128, D], row 128*t + p
        out[b, h * QS : (h + 1) * QS, :] = (
            oc.transpose(1, 0, 2).reshape(QS, D)
        )
    return out


# revision 27
# speedup vs baseline: 1.0509x; 1.0509x over previous
"""Sliding-window attention (window = [i-128, i+128]) on 8 TRN2 NeuronCores.

Problem: B=4, L=4096, D=256, fp32.  out = softmax(mask(Q K^T / sqrt(256))) V
with the softmax restricted to keys j in [i-128, i+128] for query i.

Sharding (no collectives): core c handles (batch b = c//2, query-half
h = c%2) -> 2048 queries with a 2304-row K/V halo slab; rows outside
[0, L) are zero-padded and carry a 0 validity indicator that removes them
from the softmax denominator.

Per-core kernel, S^T layout (keys on partitions, queries on free dim):
  - 8 superblocks of 256 queries; each sees a 512-key window (4 chunks of
    128 keys). Chunk 0 is only valid for queries r<128 and chunk 3 only
    for r>=128, so they share one PSUM slot ("folded" layout) and mm1
    computes only their valid query halves.
  - mm1: S^T = K Q^T accumulated over 2 d-chunks into PSUM [128, 3, 256].
  - exp: one ACT pass Exp(S^T / 16) -> SBUF (bf16/f32 per variant).
  - band mask (keep iff 0 <= jf - r <= 256): 4 GPSIMD affine_selects.
  - mm2: O = sum_chunks P_chunk^T @ V_chunk; V carries an extra indicator
    column whose matmul output is the softmax denominator.
  - normalize: DVE reciprocal + broadcast multiply (second subblock on
    the ACT engine to balance load); outputs batched 2 superblocks per
    DMA (last two solo for a faster tail).

Measured on trn2 (8 cores, NTFF profile): ~38 us NEFF exec, scale-
relative max abs error 4.4e-3 vs the fp32 reference (bf16 variant).
KERNEL_VARIANT=f32r gives 2.2e-4 error at ~63 us; f32 gives 3.4e-6 at
~83 us.
"""

import os

import numpy as np

import concourse.bass as bass  # noqa: F401  (engine types via nc)
import concourse.mybir as mybir
import concourse.tile as tile
from concourse import bacc
from concourse.bass_utils import run_bass_kernel_spmd

B = 4
L = 4096
D = 256
LW = 128                 # window half-width
N_CORES = 8
QS = L // 2              # queries per core
KS = QS + 2 * LW         # k/v slab rows per core
SB = 256                 # superblock query count
NSB = QS // SB           # superblocks per core
NKC = KS // 128          # key chunks per core
VW = 258                 # V width: 256 data + 1 indicator + 1 pad

_F32 = mybir.dt.float32

VARIANT = os.environ.get("KERNEL_VARIANT", "bf16")

# input piece boundaries for streamed DMA (few, large, arrival-ordered)
QT_PIECES = [(0, 1), (1, 2), (2, 4), (4, 6), (6, 8)]      # superblock ranges
KV_PIECES = [(0, 4), (4, 8), (8, 12), (12, 15), (15, 18)]  # key-chunk ranges


def build_bass(variant=VARIANT):
    """variant: 'f32' (exact), 'f32r' (fp32 bits, fast reduced-precision
    matmul), 'bf16' (bf16 inputs + probabilities, fp32 accumulation)."""
    if variant == "bf16":
        mm_dtype = mybir.dt.bfloat16
    elif variant == "f32r":
        mm_dtype = mybir.dt.float32r
    else:
        mm_dtype = _F32

    nc = bacc.Bacc(
        "TRN2", target_bir_lowering=False, debug=False, num_devices=N_CORES
    )
    qT = nc.declare_dram_parameter("qT", [128, NSB, 2, SB], mm_dtype, isOutput=False)
    kT = nc.declare_dram_parameter("kT", [128, NKC, 2, 128], mm_dtype, isOutput=False)
    vA = nc.declare_dram_parameter("vA", [128, NKC, VW], mm_dtype, isOutput=False)
    out = nc.declare_dram_parameter("out", [128, QS // 128, D], _F32, isOutput=True)

    inv_sqrt_d = float(1.0 / np.sqrt(D))

    with tile.TileContext(nc) as tc:
        with (
            tc.tile_pool(name="res", bufs=1) as res,
            tc.tile_pool(name="work", bufs=4) as work,
            tc.tile_pool(name="outp", bufs=2) as outp,
            tc.tile_pool(name="rcp", bufs=4) as rcp,
            tc.tile_pool(name="ps_s", bufs=2, space="PSUM") as ps_s,
            tc.tile_pool(name="ps_o", bufs=4, space="PSUM") as ps_o,
        ):
            qT_sb = res.tile([128, NSB, 2, SB], mm_dtype)
            kT_sb = res.tile([128, NKC, 2, 128], mm_dtype)
            vA_sb = res.tile([128, NKC, VW], mm_dtype)

            # streamed input DMA (~0.65us issue cost per instruction,
            # FIFO per HWDGE ring): mm1 operands interleaved on the sync
            # ring, mm2 operands (vA, needed later) on the scalar ring.
            first = True
            for (k0, k1), (s0, s1) in zip(KV_PIECES, QT_PIECES):
                if first:
                    # qT piece 0 gates the first real matmul - issue it first
                    nc.sync.dma_start(qT_sb[:, s0:s1], qT[:, s0:s1])
                    nc.sync.dma_start(kT_sb[:, k0:k1], kT[:, k0:k1])
                    first = False
                else:
                    nc.sync.dma_start(kT_sb[:, k0:k1], kT[:, k0:k1])
                    nc.sync.dma_start(qT_sb[:, s0:s1], qT[:, s0:s1])
            for k0, k1 in [(0, 3), (3, 8), (8, 13), (13, 18)]:
                nc.scalar.dma_start(vA_sb[:, k0:k1], vA[:, k0:k1])

            # PE warm-up: dummy matmuls while input DMAs land, so the HAM
            # clock-gate reaches 8/8 soon after the real matmuls start;
            # the real stream continues the busy window seamlessly.
            warm_t = res.tile([128, 128], mm_dtype)
            nc.vector.memset(warm_t[:], 0.0)
            warm_ps = ps_o.tile([128, 128], _F32, tag="psum_o")
            for _ in range(100):
                nc.tensor.matmul(
                    warm_ps[:, 0:64], lhsT=warm_t[:], rhs=warm_t[:, 0:64],
                    start=True, stop=True,
                )

            # Reusable band masks: m0 covers the folded slot0 (chunk0 for
            # r<128, chunk3 for r>=128) applied by GPSIMD affine_selects
            # in-place on exp_s; m12 covers slots 1+2, applied as ONE DVE
            # multiply [128, 2, SB]. Mask conditions (keep iff >= 0):
            #  slot0[:, 0:128]  (chunk0): p - r
            #  slot0[:, 128:]   (chunk3): (r-128) - p
            #  slot1 (chunk1): 128 + p - r
            #  slot2 (chunk2): r - p
            m12 = res.tile([128, 2, SB], mm_dtype)
            nc.vector.memset(m12[:], 1.0)
            for (slot, sl), base, cm, step, n in [
                ((0, slice(0, SB)), 128, 1, -1, SB),
                ((1, slice(0, SB)), 0, -1, 1, SB),
            ]:
                nc.gpsimd.affine_select(
                    out=m12[:, slot, sl],
                    in_=m12[:, slot, sl],
                    compare_op=mybir.AluOpType.is_ge,
                    fill=0.0,
                    base=base,
                    channel_multiplier=cm,
                    pattern=[[step, n]],
                )

            def emit_mm1_exp(s):
                # mm1 into folded PSUM [128, 3, SB]: slot0 holds chunk0
                # (queries 0:128) and chunk3 (queries 128:256).
                psum_s = ps_s.tile([128, 3, SB], _F32)
                for jj, q_sl, slot, p_sl in [
                    (0, slice(0, 128), 0, slice(0, 128)),        # chunk 0
                    (1, slice(0, SB), 1, slice(0, SB)),          # chunk 1
                    (2, slice(0, SB), 2, slice(0, SB)),          # chunk 2
                    (3, slice(128, SB), 0, slice(128, SB)),      # chunk 3
                ]:
                    jc = 2 * s + jj
                    for dc in range(2):
                        nc.tensor.matmul(
                            psum_s[:, slot, p_sl],
                            lhsT=kT_sb[:, jc, dc, :],
                            rhs=qT_sb[:, s, dc, q_sl],
                            start=(dc == 0),
                            stop=(dc == 1),
                        )

                # exp over the folded window in one ACT pass, then band
                # masks: folded slot0 halves on GPSIMD, slots 1+2 as one
                # DVE multiply (GPSIMD and DVE run concurrently).
                exp_s = work.tile([128, 3, SB], mm_dtype)
                nc.scalar.activation(
                    exp_s[:],
                    psum_s[:],
                    mybir.ActivationFunctionType.Exp,
                    scale=inv_sqrt_d,
                )
                for (slot, sl), base, cm, step, n in [
                    ((0, slice(0, 128)), 0, 1, -1, 128),
                    ((0, slice(128, SB)), 0, -1, 1, 128),
                ]:
                    nc.gpsimd.affine_select(
                        out=exp_s[:, slot, sl],
                        in_=exp_s[:, slot, sl],
                        compare_op=mybir.AluOpType.is_ge,
                        fill=0.0,
                        base=base,
                        channel_multiplier=cm,
                        pattern=[[step, n]],
                    )
                nc.vector.tensor_mul(
                    out=exp_s[:, 1:3, :], in0=exp_s[:, 1:3, :], in1=m12[:]
                )
                return exp_s

            o_hold = {}

            def emit_mm2(s, exp_s):
                single_out = s >= 6  # faster tail: last 2 superblocks solo
                if single_out:
                    o_sb = outp.tile([128, 2, D], _F32, tag="o_single")
                elif s % 2 == 0:
                    o_sb = outp.tile([128, 4, D], _F32, tag="o_pair")
                    o_hold[0] = o_sb
                else:
                    o_sb = o_hold.pop(0)
                for qc in range(2):
                    if qc == 0:
                        parts = [(0, 0, slice(0, 128)), (1, 1, slice(0, 128)),
                                 (2, 2, slice(0, 128))]
                    else:
                        parts = [(1, 1, slice(128, SB)), (2, 2, slice(128, SB)),
                                 (3, 0, slice(128, SB))]
                    psum_o = ps_o.tile([128, VW], _F32)
                    for i, (jj, slot, q_sl) in enumerate(parts):
                        nc.tensor.matmul(
                            psum_o[:],
                            lhsT=exp_s[:, slot, q_sl],
                            rhs=vA_sb[:, 2 * s + jj, :],
                            start=(i == 0),
                            stop=(i == 2),
                        )
                    recip = rcp.tile([128, 1], _F32)
                    nc.vector.reciprocal(recip[:], psum_o[:, 256:257])
                    oc = qc if single_out else 2 * (s % 2) + qc
                    if qc == 0:
                        nc.vector.tensor_scalar_mul(
                            o_sb[:, oc, :], psum_o[:, 0:D], recip[:]
                        )
                    else:
                        # second subblock normalized on ACT to offload DVE
                        nc.scalar.mul(o_sb[:, oc, :], psum_o[:, 0:D], recip[:])
                if single_out:
                    t0 = 2 * s
                    nc.sync.dma_start(out[:, t0 : t0 + 2, :], o_sb[:])
                elif s % 2 == 1:
                    t0 = 2 * (s - 1)
                    nc.sync.dma_start(out[:, t0 : t0 + 4, :], o_sb[:])

            # depth-2 software pipeline: PE FIFO runs mm1(s) two
            # superblocks ahead of mm2(s), so the exp->mask chain (ACT +
            # GPSIMD/DVE, ~1.6us) never stalls the PE.
            exp_tiles = {}
            for s in range(NSB):
                exp_tiles[s] = emit_mm1_exp(s)
                if s >= 2:
                    emit_mm2(s - 2, exp_tiles.pop(s - 2))
            emit_mm2(NSB - 2, exp_tiles.pop(NSB - 2))
            emit_mm2(NSB - 1, exp_tiles.pop(NSB - 1))

    nc.compile()
    return nc


def make_in_maps(query, key, value, np_dtype=np.float32):
    """Host-side shard + transpose + pad. Returns list of 8 input dicts."""
    in_maps = []
    for c in range(N_CORES):
        b, h = c // 2, c % 2
        q0 = h * QS
        qc = np.asarray(query[b, q0 : q0 + QS, :], dtype=np.float32)
        # qT[p, s, dc, r] = qc[SB*s + r, 128*dc + p]
        qT = np.ascontiguousarray(
            qc.reshape(NSB, SB, 2, 128).transpose(3, 0, 2, 1)
        ).astype(np_dtype)

        kstart = q0 - LW
        lo, hi = max(0, kstart), min(L, kstart + KS)
        kp = np.zeros((KS, D), np.float32)
        kp[lo - kstart : hi - kstart] = key[b, lo:hi]
        # kT[p, jc, dc, j] = kp[128*jc + j, 128*dc + p]
        kT = np.ascontiguousarray(
            kp.reshape(NKC, 128, 2, 128).transpose(3, 0, 2, 1)
        ).astype(np_dtype)

        va = np.zeros((KS, VW), np.float32)
        va[lo - kstart : hi - kstart, :D] = value[b, lo:hi]
        va[lo - kstart : hi - kstart, D] = 1.0
        vA = np.ascontiguousarray(
            va.reshape(NKC, 128, VW).transpose(1, 0, 2)
        ).astype(np_dtype)

        in_maps.append({"qT": qT, "kT": kT, "vA": vA})
    return in_maps


_NC_CACHE = {}


def _get_nc():
    if "nc" not in _NC_CACHE:
        _NC_CACHE["nc"] = build_bass(VARIANT)
    return _NC_CACHE["nc"]


def _np_in_dtype():
    if VARIANT == "bf16":
        import ml_dtypes

        return ml_dtypes.bfloat16
    return np.float32


def kernel(query, key, value):
    nc = _get_nc()
    in_maps = make_in_maps(query, key, value, np_dtype=_np_in_dtype())
    res = run_bass_kernel_spmd(nc, in_maps, core_ids=list(range(N_CORES)))
    out = np.empty((B, L, D), np.float32)
    for c in range(N_CORES):
        b, h = c // 2, c % 2
        oc = res.results[c]["out"]  # [128, QS//128, D], row 128*t + p
        out[b, h * QS : (h + 1) * QS, :] = (
            oc.transpose(1, 0, 2).reshape(QS, D)
        )
    return out
